# revision 3
# baseline (speedup 1.0000x reference)
"""Trainium2 Bass kernel for nn_CombineGraph (GCE-GNN LocalAggregator).

Computation (per batch b):
    h = emb_table[inputs[b]]                         # [L, D]
    e_k[i,j] = leakyrelu(sum_d h[i,d]*h[j,d]*a_k[d]) # 4 edge-type logits
    alpha = softmax_j(select-by-adj(e_k), -9e15 fill)
    out[b] = alpha @ h

Sharding: pure data-parallel over batch B=512 across 8 NeuronCores
(64 batches/core). emb_table + a-vectors replicated; no collectives.

Device algorithm per batch (transposed-softmax formulation):
  - indirect-DMA gather h' = emb_aug[idx] -> [100, 129] (col 128 == 1.0,
    pre-appended on host) in SBUF.
  - PE transpose: hT [128,100] (D on partitions).
  - scaled[:, k*100:+100] = hT * a_k  (DVE tensor_scalar, per-partition scalar)
  - e = hT.T @ scaled -> PSUM [100, 400]: e[j, k*100+i] = e_k[i,j] (symmetric!)
  - masks from adjT (gpsimd is_equal) -> m [100,400];
    w[:,0:400] = m * e (DVE); w[:,400:500] = (adjT==0)*-9e15 (gpsimd)
  - t[j,i] = sum over 5 planes (DVE grouped reduce)
  - u = LeakyRelu(t, 0.2); pT = Exp(u)   (ACT, one table set)
    (no max-subtraction needed: |logits| are tiny; masked entries
     are -9e15 -> lrelu -> -1.8e15 -> exp -> exactly 0)
  - o = pT.T @ h' -> PSUM [100,129]; col 128 = row sums s[i]
  - out = o[:, 0:128] * (1/s)  (DVE reciprocal + tensor_scalar) -> DMA out
"""
import numpy as np

import concourse.bass as bass
import concourse.bacc as bacc
import concourse.tile as tile
from concourse import mybir
from concourse import bass_utils
from concourse.masks import make_identity

try:
    import ml_dtypes
    _BF16 = ml_dtypes.bfloat16
except ImportError:  # pragma: no cover
    import jax.numpy as jnp
    _BF16 = jnp.bfloat16

B, L, D, V = 512, 100, 128, 200000
NCORES = 8
BS = B // NCORES          # 64 batches per core
NB = 8                    # batches per adj DMA group
NEG = -9e15
NEG_SLOPE = 0.2
DA = D + 4                # h tile free size (129 used, pad to 132)


def build_nc(reps: int = 1):
    """Build + compile the per-core Bass program (SPMD, shared by all cores).

    reps>1 wraps the whole 64-batch body in a hardware loop (for timing)."""
    nc = bacc.Bacc("TRN2", target_bir_lowering=False, debug=False,
                   enable_asserts=False, num_devices=NCORES)
    f32 = mybir.dt.float32
    bf16 = mybir.dt.bfloat16
    i32 = mybir.dt.int32

    emb = nc.dram_tensor("emb", [V, D + 1], f32, kind="ExternalInput")
    idx_t = nc.dram_tensor("idx_t", [L, BS], i32, kind="ExternalInput")
    adj_t = nc.dram_tensor("adj_t", [L, BS, L], bf16, kind="ExternalInput")
    a_pat = nc.dram_tensor("a_pat", [D, 4 * L], f32, kind="ExternalInput")
    out_d = nc.dram_tensor("out", [BS, L, D], f32, kind="ExternalOutput")

    from contextlib import ExitStack
    with tile.TileContext(nc) as tc, ExitStack() as ctx:
        cp = ctx.enter_context(tc.tile_pool(name="const", bufs=1))
        adj_pool = ctx.enter_context(tc.tile_pool(name="adj", bufs=2))
        sb = ctx.enter_context(tc.tile_pool(name="sb", bufs=3))
        ps_hT = ctx.enter_context(tc.tile_pool(name="ps_hT", bufs=2,
                                               space="PSUM"))
        ps_e = ctx.enter_context(tc.tile_pool(name="ps_e", bufs=2,
                                              space="PSUM"))
        ps_o = ctx.enter_context(tc.tile_pool(name="ps_o", bufs=2,
                                              space="PSUM"))

        idx_sb = cp.tile([L, BS], i32)
        nc.sync.dma_start(out=idx_sb[:], in_=idx_t.ap())
        a_sb = cp.tile([D, 4 * L], f32)
        nc.sync.dma_start(out=a_sb[:], in_=a_pat.ap())
        ident = cp.tile([L, L], f32)
        make_identity(nc, ident[:])

        def body(_iv=None):
            for n in range(BS):
                grp, nn = divmod(n, NB)
                if nn == 0:
                    adj_new = adj_pool.tile([L, NB, L], bf16, tag="adj")
                    body.adj_sb = adj_new
                    nc.sync.dma_start(
                        out=adj_new[:],
                        in_=adj_t.ap()[:, grp * NB:(grp + 1) * NB, :])
                adj_sb = body.adj_sb
                adjn = adj_sb[:, nn, :]

                # gather h' (with ones column at 128)
                h = sb.tile([L, DA], f32, tag="h")
                nc.gpsimd.indirect_dma_start(
                    out=h[:, 0:D + 1], out_offset=None, in_=emb.ap(),
                    in_offset=bass.IndirectOffsetOnAxis(
                        ap=idx_sb[:, n:n + 1], axis=0))

                # hT = h.T (PE), evac to SBUF
                hT_ps = ps_hT.tile([D, L], f32, tag="hT_ps")
                nc.tensor.transpose(out=hT_ps[:], in_=h[:, 0:D],
                                    identity=ident[:])
                hT = sb.tile([D, L], f32, tag="hT")
                nc.vector.tensor_copy(hT[:], hT_ps[:])

                # scaled[:, k] = hT * a_k
                scaled = sb.tile([D, 4 * L], f32, tag="scaled")
                for k in range(4):
                    nc.vector.tensor_scalar(
                        out=scaled[:, k * L:(k + 1) * L], in0=hT[:],
                        scalar1=a_sb[:, k * L:k * L + 1], scalar2=None,
                        op0=mybir.AluOpType.mult)

                # e[j, k*100+i] = e_k (symmetric)
                e_ps = ps_e.tile([L, 4 * L], f32, tag="e_ps")
                nc.tensor.matmul(out=e_ps[:], lhsT=hT[:], rhs=scaled[:],
                                 start=True, stop=True)

                # masks + select + -inf fill
                m = sb.tile([L, 4 * L], f32, tag="m")
                for k in range(4):
                    nc.gpsimd.tensor_scalar(
                        out=m[:, k * L:(k + 1) * L], in0=adjn,
                        scalar1=float(k + 1), scalar2=None,
                        op0=mybir.AluOpType.is_equal)
                w = sb.tile([L, 5 * L], f32, tag="w")
                nc.vector.tensor_tensor(out=w[:, 0:4 * L], in0=m[:],
                                        in1=e_ps[:],
                                        op=mybir.AluOpType.mult)
                nc.gpsimd.tensor_scalar(
                    out=w[:, 4 * L:5 * L], in0=adjn,
                    scalar1=0.0, scalar2=NEG,
                    op0=mybir.AluOpType.is_equal, op1=mybir.AluOpType.mult)

                t = sb.tile([L, L], f32, tag="t")
                nc.vector.tensor_reduce(
                    out=t[:], in_=w[:].rearrange("p (k i) -> p i k", k=5),
                    axis=mybir.AxisListType.X, op=mybir.AluOpType.add)

                # pT = exp(lrelu(t))
                u = sb.tile([L, L], f32, tag="u")
                nc.scalar.activation(out=u[:], in_=t[:],
                                     func=mybir.ActivationFunctionType.Lrelu,
                                     alpha=NEG_SLOPE)
                pT = sb.tile([L, L], f32, tag="pT")
                nc.scalar.activation(out=pT[:], in_=u[:],
                                     func=mybir.ActivationFunctionType.Exp)

                # out rows + row-sums in one matmul (ones column)
                o_ps = ps_o.tile([L, D + 1], f32, tag="o_ps")
                nc.tensor.matmul(out=o_ps[:], lhsT=pT[:], rhs=h[:, 0:D + 1],
                                 start=True, stop=True)

                r = sb.tile([L, 1], f32, tag="r")
                nc.vector.reciprocal(r[:], o_ps[:, D:D + 1])
                o_sb = sb.tile([L, D], f32, tag="o_sb")
                nc.vector.tensor_scalar(out=o_sb[:], in0=o_ps[:, 0:D],
                                        scalar1=r[:, 0:1], scalar2=None,
                                        op0=mybir.AluOpType.mult)
                nc.sync.dma_start(out=out_d.ap()[n], in_=o_sb[:])

        if reps == 1:
            body()
        else:
            with tc.For_i(0, reps, 1) as iv:
                body(iv)

    nc.compile()
    return nc


_CACHED_NC = None


def _shard_inputs(inputs, adj, emb_table, a0, a1, a2, a3):
    inputs = np.asarray(inputs).astype(np.int32)
    adj = np.asarray(adj)
    emb_table = np.asarray(emb_table, dtype=np.float32)
    avecs = [np.asarray(a, dtype=np.float32) for a in (a0, a1, a2, a3)]

    emb_aug = np.concatenate(
        [emb_table, np.ones((V, 1), np.float32)], axis=1)   # [V, 129]
    a_pat = np.concatenate(
        [np.tile(a[:, None], (1, L)) for a in avecs], axis=1)  # [128, 400]

    in_maps = []
    for c in range(NCORES):
        sl = slice(c * BS, (c + 1) * BS)
        idx_c = np.ascontiguousarray(inputs[sl].T)                 # [L, BS]
        adj_c = np.ascontiguousarray(
            adj[sl].transpose(2, 0, 1)).astype(_BF16)              # [L,BS,L]
        in_maps.append(dict(emb=emb_aug, idx_t=idx_c, adj_t=adj_c,
                            a_pat=a_pat))
    return in_maps


def kernel(inputs, adj, mask_item, item, emb_table, a0, a1, a2, a3):
    """Full inputs in, full output out. mask_item/item are unused by the
    reference model's forward pass."""
    global _CACHED_NC
    if _CACHED_NC is None:
        _CACHED_NC = build_nc(reps=1)
    nc = _CACHED_NC

    in_maps = _shard_inputs(inputs, adj, emb_table, a0, a1, a2, a3)
    res = bass_utils.run_bass_kernel_spmd(nc, in_maps,
                                          core_ids=list(range(NCORES)))
    out = np.concatenate([np.asarray(res.results[c]["out"])
                          for c in range(NCORES)], axis=0)
    return out


# revision 4
# speedup vs baseline: 2.3178x; 2.3178x over previous
"""Trainium2 Bass kernel for nn_CombineGraph (GCE-GNN LocalAggregator).

Computation (per batch b):
    h = emb_table[inputs[b]]                         # [L, D]
    e_k[i,j] = leakyrelu(sum_d h[i,d]*h[j,d]*a_k[d]) # 4 edge-type logits
    alpha = softmax_j(select-by-adj(e_k), -9e15 fill)
    out[b] = alpha @ h

Sharding: pure data-parallel over batch B=512 across 8 NeuronCores
(64 batches/core). emb_table + a-vectors replicated; no collectives.

Device algorithm per batch (transposed-softmax formulation):
  - indirect-DMA gather h' = emb_aug[idx] -> [100, 129] (col 128 == 1.0,
    pre-appended on host) in SBUF.
  - PE transpose: hT [128,100] (D on partitions).
  - scaled[:, k*100:+100] = hT * a_k  (DVE tensor_scalar, per-partition scalar)
  - e = hT.T @ scaled -> PSUM [100, 400]: e[j, k*100+i] = e_k[i,j] (symmetric!)
  - masks from adjT (gpsimd is_equal) -> m [100,400];
    w[:,0:400] = m * e (DVE); w[:,400:500] = (adjT==0)*-9e15 (gpsimd)
  - t[j,i] = sum over 5 planes (DVE grouped reduce)
  - u = LeakyRelu(t, 0.2); pT = Exp(u)   (ACT, one table set)
    (no max-subtraction needed: |logits| are tiny; masked entries
     are -9e15 -> lrelu -> -1.8e15 -> exp -> exactly 0)
  - o = pT.T @ h' -> PSUM [100,129]; col 128 = row sums s[i]
  - out = o[:, 0:128] * (1/s)  (DVE reciprocal + tensor_scalar) -> DMA out
"""
import numpy as np

import concourse.bass as bass
import concourse.bacc as bacc
import concourse.tile as tile
from concourse import mybir
from concourse import bass_utils
from concourse.masks import make_identity

try:
    import ml_dtypes
    _BF16 = ml_dtypes.bfloat16
except ImportError:  # pragma: no cover
    import jax.numpy as jnp
    _BF16 = jnp.bfloat16

B, L, D, V = 512, 100, 128, 200000
NCORES = 8
BS = B // NCORES          # 64 batches per core
NB = 8                    # batches per adj DMA group
NEG = -9e15
NEG_SLOPE = 0.2
DA = D + 4                # h tile free size (129 used, pad to 132)


def build_nc(reps: int = 1):
    """Build + compile the per-core Bass program (SPMD, shared by all cores).

    reps>1 wraps the whole 64-batch body in a hardware loop (for timing)."""
    nc = bacc.Bacc("TRN2", target_bir_lowering=False, debug=False,
                   enable_asserts=False, num_devices=NCORES)
    f32 = mybir.dt.float32
    bf16 = mybir.dt.bfloat16
    i32 = mybir.dt.int32

    emb = nc.dram_tensor("emb", [V, D + 1], f32, kind="ExternalInput")
    idx_t = nc.dram_tensor("idx_t", [L, BS], i32, kind="ExternalInput")
    adj_t = nc.dram_tensor("adj_t", [L, BS, L], bf16, kind="ExternalInput")
    a_pat = nc.dram_tensor("a_pat", [D, 4 * L], f32, kind="ExternalInput")
    out_d = nc.dram_tensor("out", [BS, L, D], f32, kind="ExternalOutput")

    from contextlib import ExitStack
    with tile.TileContext(nc) as tc, ExitStack() as ctx:
        cp = ctx.enter_context(tc.tile_pool(name="const", bufs=1))
        adj_pool = ctx.enter_context(tc.tile_pool(name="adj", bufs=2))
        sb = ctx.enter_context(tc.tile_pool(name="sb", bufs=3))
        ps_hT = ctx.enter_context(tc.tile_pool(name="ps_hT", bufs=2,
                                               space="PSUM"))
        ps_e = ctx.enter_context(tc.tile_pool(name="ps_e", bufs=2,
                                              space="PSUM"))
        ps_o = ctx.enter_context(tc.tile_pool(name="ps_o", bufs=2,
                                              space="PSUM"))

        idx_sb = cp.tile([L, BS], i32)
        nc.sync.dma_start(out=idx_sb[:], in_=idx_t.ap())
        a_sb = cp.tile([D, 4 * L], f32)
        nc.sync.dma_start(out=a_sb[:], in_=a_pat.ap())
        ident = cp.tile([L, L], f32)
        make_identity(nc, ident[:])
        kpat5 = cp.tile([L, 5 * L], bf16)
        for k in range(5):
            nc.gpsimd.memset(kpat5[:, k * L:(k + 1) * L],
                             float(k + 1) if k < 4 else 0.0)
        negc = cp.tile([L, L], f32)
        nc.gpsimd.memset(negc[:], NEG)

        def body(_iv=None):
            for n in range(BS):
                grp, nn = divmod(n, NB)
                if nn == 0:
                    adj_new = adj_pool.tile([L, NB, L], bf16, tag="adj")
                    body.adj_sb = adj_new
                    nc.sync.dma_start(
                        out=adj_new[:],
                        in_=adj_t.ap()[:, grp * NB:(grp + 1) * NB, :])
                adj_sb = body.adj_sb
                adjn = adj_sb[:, nn, :]

                # gather h' (with ones column at 128)
                h = sb.tile([L, DA], f32, tag="h")
                nc.gpsimd.indirect_dma_start(
                    out=h[:, 0:D + 1], out_offset=None, in_=emb.ap(),
                    in_offset=bass.IndirectOffsetOnAxis(
                        ap=idx_sb[:, n:n + 1], axis=0))

                # hT = h.T (PE), evac to SBUF
                hT_ps = ps_hT.tile([D, L], f32, tag="hT_ps")
                nc.tensor.transpose(out=hT_ps[:], in_=h[:, 0:D],
                                    identity=ident[:])
                hT = sb.tile([D, L], f32, tag="hT")
                nc.vector.tensor_copy(hT[:], hT_ps[:])

                # scaled[:, k] = hT * a_k
                scaled = sb.tile([D, 4 * L], f32, tag="scaled")
                nc.vector.tensor_tensor(
                    out=scaled[:].rearrange("p (k i) -> p k i", k=4),
                    in0=hT[:].unsqueeze(1).to_broadcast([D, 4, L]),
                    in1=a_sb[:].rearrange("p (k i) -> p k i", k=4),
                    op=mybir.AluOpType.mult)

                # e[j, k*100+i] = e_k (symmetric)
                e_ps = ps_e.tile([L, 4 * L], f32, tag="e_ps")
                nc.tensor.matmul(out=e_ps[:], lhsT=hT[:], rhs=scaled[:],
                                 start=True, stop=True)

                # masks + select + -inf fill
                m5 = sb.tile([L, 5 * L], bf16, tag="m5")
                nc.vector.tensor_tensor(
                    out=m5[:].rearrange("p (k i) -> p k i", k=5),
                    in0=adjn.unsqueeze(1).to_broadcast([L, 5, L]),
                    in1=kpat5[:].rearrange("p (k i) -> p k i", k=5),
                    op=mybir.AluOpType.is_equal)
                w = sb.tile([L, 5 * L], f32, tag="w")
                nc.vector.tensor_tensor(out=w[:, 0:4 * L], in0=m5[:, 0:4 * L],
                                        in1=e_ps[:],
                                        op=mybir.AluOpType.mult)
                nc.vector.tensor_tensor(out=w[:, 4 * L:5 * L],
                                        in0=m5[:, 4 * L:5 * L], in1=negc[:],
                                        op=mybir.AluOpType.mult)

                t = sb.tile([L, L], f32, tag="t")
                nc.vector.tensor_reduce(
                    out=t[:], in_=w[:].rearrange("p (k i) -> p i k", k=5),
                    axis=mybir.AxisListType.X, op=mybir.AluOpType.add)

                # pT = exp(lrelu(t))
                u = sb.tile([L, L], f32, tag="u")
                nc.scalar.activation(out=u[:], in_=t[:],
                                     func=mybir.ActivationFunctionType.Lrelu,
                                     alpha=NEG_SLOPE)
                pT = sb.tile([L, L], f32, tag="pT")
                nc.scalar.activation(out=pT[:], in_=u[:],
                                     func=mybir.ActivationFunctionType.Exp)

                # out rows + row-sums in one matmul (ones column)
                o_ps = ps_o.tile([L, D + 1], f32, tag="o_ps")
                nc.tensor.matmul(out=o_ps[:], lhsT=pT[:], rhs=h[:, 0:D + 1],
                                 start=True, stop=True)

                r = sb.tile([L, 1], f32, tag="r")
                nc.vector.reciprocal(r[:], o_ps[:, D:D + 1])
                o_sb = sb.tile([L, D], f32, tag="o_sb")
                nc.vector.tensor_scalar(out=o_sb[:], in0=o_ps[:, 0:D],
                                        scalar1=r[:, 0:1], scalar2=None,
                                        op0=mybir.AluOpType.mult)
                nc.sync.dma_start(out=out_d.ap()[n], in_=o_sb[:])

        if reps == 1:
            body()
        else:
            with tc.For_i(0, reps, 1) as iv:
                body(iv)

    nc.compile()
    return nc


_CACHED_NC = None


def _shard_inputs(inputs, adj, emb_table, a0, a1, a2, a3):
    inputs = np.asarray(inputs).astype(np.int32)
    adj = np.asarray(adj)
    emb_table = np.asarray(emb_table, dtype=np.float32)
    avecs = [np.asarray(a, dtype=np.float32) for a in (a0, a1, a2, a3)]

    emb_aug = np.concatenate(
        [emb_table, np.ones((V, 1), np.float32)], axis=1)   # [V, 129]
    a_pat = np.concatenate(
        [np.tile(a[:, None], (1, L)) for a in avecs], axis=1)  # [128, 400]

    in_maps = []
    for c in range(NCORES):
        sl = slice(c * BS, (c + 1) * BS)
        idx_c = np.ascontiguousarray(inputs[sl].T)                 # [L, BS]
        adj_c = np.ascontiguousarray(
            adj[sl].transpose(2, 0, 1)).astype(_BF16)              # [L,BS,L]
        in_maps.append(dict(emb=emb_aug, idx_t=idx_c, adj_t=adj_c,
                            a_pat=a_pat))
    return in_maps


def kernel(inputs, adj, mask_item, item, emb_table, a0, a1, a2, a3):
    """Full inputs in, full output out. mask_item/item are unused by the
    reference model's forward pass."""
    global _CACHED_NC
    if _CACHED_NC is None:
        _CACHED_NC = build_nc(reps=1)
    nc = _CACHED_NC

    in_maps = _shard_inputs(inputs, adj, emb_table, a0, a1, a2, a3)
    res = bass_utils.run_bass_kernel_spmd(nc, in_maps,
                                          core_ids=list(range(NCORES)))
    out = np.concatenate([np.asarray(res.results[c]["out"])
                          for c in range(NCORES)], axis=0)
    return out


# revision 5
# speedup vs baseline: 2.7442x; 1.1839x over previous
"""Trainium2 Bass kernel for nn_CombineGraph (GCE-GNN LocalAggregator).

Computation (per batch b):
    h = emb_table[inputs[b]]                         # [L, D]
    e_k[i,j] = leakyrelu(sum_d h[i,d]*h[j,d]*a_k[d]) # 4 edge-type logits
    alpha = softmax_j(select-by-adj(e_k), -9e15 fill)
    out[b] = alpha @ h

Sharding: pure data-parallel over batch B=512 across 8 NeuronCores
(64 batches/core). emb_table + a-vectors replicated; no collectives.

Device algorithm per batch (transposed-softmax formulation):
  - indirect-DMA gather h' = emb_aug[idx] -> [100, 129] (col 128 == 1.0,
    pre-appended on host) in SBUF.
  - PE transpose: hT [128,100] (D on partitions).
  - scaled[:, k*100:+100] = hT * a_k  (DVE tensor_scalar, per-partition scalar)
  - e = hT.T @ scaled -> PSUM [100, 400]: e[j, k*100+i] = e_k[i,j] (symmetric!)
  - masks from adjT (gpsimd is_equal) -> m [100,400];
    w[:,0:400] = m * e (DVE); w[:,400:500] = (adjT==0)*-9e15 (gpsimd)
  - t[j,i] = sum over 5 planes (DVE grouped reduce)
  - u = LeakyRelu(t, 0.2); pT = Exp(u)   (ACT, one table set)
    (no max-subtraction needed: |logits| are tiny; masked entries
     are -9e15 -> lrelu -> -1.8e15 -> exp -> exactly 0)
  - o = pT.T @ h' -> PSUM [100,129]; col 128 = row sums s[i]
  - out = o[:, 0:128] * (1/s)  (DVE reciprocal + tensor_scalar) -> DMA out
"""
import numpy as np

import concourse.bass as bass
import concourse.bacc as bacc
import concourse.tile as tile
from concourse import mybir
from concourse import bass_utils
from concourse.masks import make_identity

try:
    import ml_dtypes
    _BF16 = ml_dtypes.bfloat16
except ImportError:  # pragma: no cover
    import jax.numpy as jnp
    _BF16 = jnp.bfloat16

B, L, D, V = 512, 100, 128, 200000
NCORES = 8
BS = B // NCORES          # 64 batches per core
NB = 8                    # batches per adj DMA group
NEG = -9e15
NEG_SLOPE = 0.2
DA = D + 4                # h tile free size (129 used, pad to 132)


def build_nc(reps: int = 1):
    """Build + compile the per-core Bass program (SPMD, shared by all cores).

    reps>1 wraps the whole 64-batch body in a hardware loop (for timing)."""
    nc = bacc.Bacc("TRN2", target_bir_lowering=False, debug=False,
                   enable_asserts=False, num_devices=NCORES)
    f32 = mybir.dt.float32
    bf16 = mybir.dt.bfloat16
    i32 = mybir.dt.int32

    emb = nc.dram_tensor("emb", [V, D + 1], f32, kind="ExternalInput")
    idx_t = nc.dram_tensor("idx_t", [L, BS], i32, kind="ExternalInput")
    adj_t = nc.dram_tensor("adj_t", [L, BS, L], bf16, kind="ExternalInput")
    a_pat = nc.dram_tensor("a_pat", [D, 4 * L], f32, kind="ExternalInput")
    out_d = nc.dram_tensor("out", [BS, L, D], f32, kind="ExternalOutput")

    from contextlib import ExitStack
    with tile.TileContext(nc) as tc, ExitStack() as ctx:
        cp = ctx.enter_context(tc.tile_pool(name="const", bufs=1))
        adj_pool = ctx.enter_context(tc.tile_pool(name="adj", bufs=2))
        sb = ctx.enter_context(tc.tile_pool(name="sb", bufs=4))
        ps_hT = ctx.enter_context(tc.tile_pool(name="ps_hT", bufs=2,
                                               space="PSUM"))
        ps_e = ctx.enter_context(tc.tile_pool(name="ps_e", bufs=3,
                                              space="PSUM"))
        ps_o = ctx.enter_context(tc.tile_pool(name="ps_o", bufs=3,
                                              space="PSUM"))

        idx_sb = cp.tile([L, BS], i32)
        nc.sync.dma_start(out=idx_sb[:], in_=idx_t.ap())
        a_sb = cp.tile([D, 4 * L], f32)
        nc.sync.dma_start(out=a_sb[:], in_=a_pat.ap())
        ident = cp.tile([L, L], f32)
        make_identity(nc, ident[:])
        kpat5 = cp.tile([L, 5 * L], bf16)
        for k in range(5):
            nc.gpsimd.memset(kpat5[:, k * L:(k + 1) * L],
                             float(k + 1) if k < 4 else 0.0)
        negc = cp.tile([L, L], f32)
        nc.gpsimd.memset(negc[:], NEG)

        def body(_iv=None):
            for n in range(BS):
                grp, nn = divmod(n, NB)
                if nn == 0:
                    adj_new = adj_pool.tile([L, NB, L], bf16, tag="adj")
                    body.adj_sb = adj_new
                    nc.sync.dma_start(
                        out=adj_new[:],
                        in_=adj_t.ap()[:, grp * NB:(grp + 1) * NB, :])
                adj_sb = body.adj_sb
                adjn = adj_sb[:, nn, :]

                # gather h' (with ones column at 128)
                h = sb.tile([L, DA], f32, tag="h")
                nc.gpsimd.indirect_dma_start(
                    out=h[:, 0:D + 1], out_offset=None, in_=emb.ap(),
                    in_offset=bass.IndirectOffsetOnAxis(
                        ap=idx_sb[:, n:n + 1], axis=0))

                # hT = h.T (PE), evac to SBUF
                hT_ps = ps_hT.tile([D, L], f32, tag="hT_ps")
                nc.tensor.transpose(out=hT_ps[:], in_=h[:, 0:D],
                                    identity=ident[:])
                hT = sb.tile([D, L], f32, tag="hT")
                nc.scalar.copy(hT[:], hT_ps[:])

                # scaled[:, k] = hT * a_k
                scaled = sb.tile([D, 4 * L], f32, tag="scaled")
                nc.vector.tensor_tensor(
                    out=scaled[:].rearrange("p (k i) -> p k i", k=4),
                    in0=hT[:].unsqueeze(1).to_broadcast([D, 4, L]),
                    in1=a_sb[:].rearrange("p (k i) -> p k i", k=4),
                    op=mybir.AluOpType.mult)

                # e[j, k*100+i] = e_k (symmetric)
                e_ps = ps_e.tile([L, 4 * L], f32, tag="e_ps")
                nc.tensor.matmul(out=e_ps[:], lhsT=hT[:], rhs=scaled[:],
                                 start=True, stop=True)

                # masks + select + -inf fill
                m5 = sb.tile([L, 5 * L], bf16, tag="m5")
                nc.vector.tensor_tensor(
                    out=m5[:].rearrange("p (k i) -> p k i", k=5),
                    in0=adjn.unsqueeze(1).to_broadcast([L, 5, L]),
                    in1=kpat5[:].rearrange("p (k i) -> p k i", k=5),
                    op=mybir.AluOpType.is_equal)
                w = sb.tile([L, 5 * L], f32, tag="w")
                nc.vector.tensor_tensor(out=w[:, 0:4 * L], in0=m5[:, 0:4 * L],
                                        in1=e_ps[:],
                                        op=mybir.AluOpType.mult)
                nc.vector.tensor_tensor(out=w[:, 4 * L:5 * L],
                                        in0=m5[:, 4 * L:5 * L], in1=negc[:],
                                        op=mybir.AluOpType.mult)

                t = sb.tile([L, L], f32, tag="t")
                nc.vector.tensor_reduce(
                    out=t[:], in_=w[:].rearrange("p (k i) -> p i k", k=5),
                    axis=mybir.AxisListType.X, op=mybir.AluOpType.add)

                # pT = exp(lrelu(t))
                u = sb.tile([L, L], f32, tag="u")
                nc.scalar.activation(out=u[:], in_=t[:],
                                     func=mybir.ActivationFunctionType.Lrelu,
                                     alpha=NEG_SLOPE)
                pT = sb.tile([L, L], f32, tag="pT")
                nc.scalar.activation(out=pT[:], in_=u[:],
                                     func=mybir.ActivationFunctionType.Exp)

                # out rows + row-sums in one matmul (ones column)
                o_ps = ps_o.tile([L, D + 1], f32, tag="o_ps")
                nc.tensor.matmul(out=o_ps[:], lhsT=pT[:], rhs=h[:, 0:D + 1],
                                 start=True, stop=True)

                r = sb.tile([L, 1], f32, tag="r")
                nc.vector.reciprocal(r[:], o_ps[:, D:D + 1])
                o_sb = sb.tile([L, D], f32, tag="o_sb")
                nc.vector.tensor_scalar(out=o_sb[:], in0=o_ps[:, 0:D],
                                        scalar1=r[:, 0:1], scalar2=None,
                                        op0=mybir.AluOpType.mult)
                nc.sync.dma_start(out=out_d.ap()[n], in_=o_sb[:])

        if reps == 1:
            body()
        else:
            with tc.For_i(0, reps, 1) as iv:
                body(iv)

    nc.compile()
    return nc


_CACHED_NC = None


def _shard_inputs(inputs, adj, emb_table, a0, a1, a2, a3):
    inputs = np.asarray(inputs).astype(np.int32)
    adj = np.asarray(adj)
    emb_table = np.asarray(emb_table, dtype=np.float32)
    avecs = [np.asarray(a, dtype=np.float32) for a in (a0, a1, a2, a3)]

    emb_aug = np.concatenate(
        [emb_table, np.ones((V, 1), np.float32)], axis=1)   # [V, 129]
    a_pat = np.concatenate(
        [np.tile(a[:, None], (1, L)) for a in avecs], axis=1)  # [128, 400]

    in_maps = []
    for c in range(NCORES):
        sl = slice(c * BS, (c + 1) * BS)
        idx_c = np.ascontiguousarray(inputs[sl].T)                 # [L, BS]
        adj_c = np.ascontiguousarray(
            adj[sl].transpose(2, 0, 1)).astype(_BF16)              # [L,BS,L]
        in_maps.append(dict(emb=emb_aug, idx_t=idx_c, adj_t=adj_c,
                            a_pat=a_pat))
    return in_maps


def kernel(inputs, adj, mask_item, item, emb_table, a0, a1, a2, a3):
    """Full inputs in, full output out. mask_item/item are unused by the
    reference model's forward pass."""
    global _CACHED_NC
    if _CACHED_NC is None:
        _CACHED_NC = build_nc(reps=1)
    nc = _CACHED_NC

    in_maps = _shard_inputs(inputs, adj, emb_table, a0, a1, a2, a3)
    res = bass_utils.run_bass_kernel_spmd(nc, in_maps,
                                          core_ids=list(range(NCORES)))
    out = np.concatenate([np.asarray(res.results[c]["out"])
                          for c in range(NCORES)], axis=0)
    return out


# revision 7
# speedup vs baseline: 2.8250x; 1.0295x over previous
"""Trainium2 Bass kernel for nn_CombineGraph (GCE-GNN LocalAggregator).

Computation (per batch b):
    h = emb_table[inputs[b]]                         # [L, D]
    e_k[i,j] = leakyrelu(sum_d h[i,d]*h[j,d]*a_k[d]) # 4 edge-type logits
    alpha = softmax_j(select-by-adj(e_k), -9e15 fill)
    out[b] = alpha @ h

Sharding: pure data-parallel over batch B=512 across 8 NeuronCores
(64 batches/core). emb_table + a-vectors replicated; no collectives.

Device algorithm per batch (transposed-softmax formulation):
  - indirect-DMA gather h' = emb_aug[idx] -> [100, 129] (col 128 == 1.0,
    pre-appended on host) in SBUF.
  - PE transpose: hT [128,100] (D on partitions).
  - scaled = hT (bcast x4) * a_pat  (one DVE broadcast tensor_tensor)
  - e = hT.T @ scaled -> PSUM [100, 400]: e[j, k*100+i] = e_k[i,j] (symmetric!)
  - m5 = (adjT bcast x5 == [1,2,3,4,0] pattern)  (one DVE is_equal; POOL is
    kept free for the indirect gathers, whose ~1.2us/op dispatch cost made
    gpsimd masks the original bottleneck)
    w[:,0:400] = m5 * e; w[:,400:500] = m5_plane4 * (-9e15)  (DVE)
  - t[j,i] = sum over 5 planes (DVE grouped reduce)
  - u = LeakyRelu(t, 0.2); pT = Exp(u)   (ACT, one table set)
    (no max-subtraction needed: |logits| are tiny; masked entries
     are -9e15 -> lrelu -> -1.8e15 -> exp -> exactly 0)
  - o = pT.T @ h' -> PSUM [100,129]; col 128 = row sums s[i]
  - out = o[:, 0:128] * (1/s)  (DVE reciprocal + tensor_scalar) -> DMA out
"""
import numpy as np

import concourse.bass as bass
import concourse.bacc as bacc
import concourse.tile as tile
from concourse import mybir
from concourse import bass_utils
from concourse.masks import make_identity

try:
    import ml_dtypes
    _BF16 = ml_dtypes.bfloat16
except ImportError:  # pragma: no cover
    import jax.numpy as jnp
    _BF16 = jnp.bfloat16

B, L, D, V = 512, 100, 128, 200000
NCORES = 8
BS = B // NCORES          # 64 batches per core
NB = 8                    # batches per adj DMA group
NEG = -9e15
NEG_SLOPE = 0.2
DA = D + 4                # h tile free size (129 used, pad to 132)


def build_nc(reps: int = 1):
    """Build + compile the per-core Bass program (SPMD, shared by all cores).

    reps>1 wraps the whole 64-batch body in a hardware loop (for timing)."""
    nc = bacc.Bacc("TRN2", target_bir_lowering=False, debug=False,
                   enable_asserts=False, num_devices=NCORES)
    f32 = mybir.dt.float32
    bf16 = mybir.dt.bfloat16
    i32 = mybir.dt.int32

    emb = nc.dram_tensor("emb", [V, D + 1], f32, kind="ExternalInput")
    idx_t = nc.dram_tensor("idx_t", [L, BS], i32, kind="ExternalInput")
    adj_t = nc.dram_tensor("adj_t", [L, BS, L], bf16, kind="ExternalInput")
    a_pat = nc.dram_tensor("a_pat", [D, 4 * L], f32, kind="ExternalInput")
    out_d = nc.dram_tensor("out", [BS, L, D], f32, kind="ExternalOutput")

    from contextlib import ExitStack
    with tile.TileContext(nc) as tc, ExitStack() as ctx:
        cp = ctx.enter_context(tc.tile_pool(name="const", bufs=1))
        adj_pool = ctx.enter_context(tc.tile_pool(name="adj", bufs=3))
        hp = ctx.enter_context(tc.tile_pool(name="hp", bufs=16))
        sb = ctx.enter_context(tc.tile_pool(name="sb", bufs=4))
        ps_hT = ctx.enter_context(tc.tile_pool(name="ps_hT", bufs=2,
                                               space="PSUM"))
        ps_e = ctx.enter_context(tc.tile_pool(name="ps_e", bufs=3,
                                              space="PSUM"))
        ps_o = ctx.enter_context(tc.tile_pool(name="ps_o", bufs=3,
                                              space="PSUM"))

        idx_sb = cp.tile([L, BS], i32)
        nc.sync.dma_start(out=idx_sb[:], in_=idx_t.ap())
        a_sb = cp.tile([D, 4 * L], f32)
        nc.sync.dma_start(out=a_sb[:], in_=a_pat.ap())
        ident = cp.tile([L, L], f32)
        make_identity(nc, ident[:])
        kpat5 = cp.tile([L, 5 * L], bf16)
        for k in range(5):
            nc.gpsimd.memset(kpat5[:, k * L:(k + 1) * L],
                             float(k + 1) if k < 4 else 0.0)
        negc = cp.tile([L, L], f32)
        nc.gpsimd.memset(negc[:], NEG)

        def body(_iv=None):
            for n in range(BS):
                grp, nn = divmod(n, NB)
                if nn == 0:
                    adj_new = adj_pool.tile([L, NB, L], bf16, tag="adj")
                    body.adj_sb = adj_new
                    nc.sync.dma_start(
                        out=adj_new[:],
                        in_=adj_t.ap()[:, grp * NB:(grp + 1) * NB, :])
                adj_sb = body.adj_sb
                adjn = adj_sb[:, nn, :]

                # gather h' (with ones column at 128)
                h = hp.tile([L, DA], f32, tag="h")
                nc.gpsimd.indirect_dma_start(
                    out=h[:, 0:D + 1], out_offset=None, in_=emb.ap(),
                    in_offset=bass.IndirectOffsetOnAxis(
                        ap=idx_sb[:, n:n + 1], axis=0))

                # hT = h.T (PE), evac to SBUF
                hT_ps = ps_hT.tile([D, L], f32, tag="hT_ps")
                nc.tensor.transpose(out=hT_ps[:], in_=h[:, 0:D],
                                    identity=ident[:])
                hT = sb.tile([D, L], f32, tag="hT")
                nc.scalar.copy(hT[:], hT_ps[:])

                # scaled[:, k] = hT * a_k
                scaled = sb.tile([D, 4 * L], f32, tag="scaled")
                nc.vector.tensor_tensor(
                    out=scaled[:].rearrange("p (k i) -> p k i", k=4),
                    in0=hT[:].unsqueeze(1).to_broadcast([D, 4, L]),
                    in1=a_sb[:].rearrange("p (k i) -> p k i", k=4),
                    op=mybir.AluOpType.mult)

                # e[j, k*100+i] = e_k (symmetric)
                e_ps = ps_e.tile([L, 4 * L], f32, tag="e_ps")
                nc.tensor.matmul(out=e_ps[:], lhsT=hT[:], rhs=scaled[:],
                                 start=True, stop=True)

                # masks + select + -inf fill
                m5 = sb.tile([L, 5 * L], bf16, tag="m5")
                nc.vector.tensor_tensor(
                    out=m5[:].rearrange("p (k i) -> p k i", k=5),
                    in0=adjn.unsqueeze(1).to_broadcast([L, 5, L]),
                    in1=kpat5[:].rearrange("p (k i) -> p k i", k=5),
                    op=mybir.AluOpType.is_equal)
                w = sb.tile([L, 5 * L], f32, tag="w")
                nc.vector.tensor_tensor(out=w[:, 0:4 * L], in0=m5[:, 0:4 * L],
                                        in1=e_ps[:],
                                        op=mybir.AluOpType.mult)
                nc.vector.tensor_tensor(out=w[:, 4 * L:5 * L],
                                        in0=m5[:, 4 * L:5 * L], in1=negc[:],
                                        op=mybir.AluOpType.mult)

                t = sb.tile([L, L], f32, tag="t")
                nc.vector.tensor_reduce(
                    out=t[:], in_=w[:].rearrange("p (k i) -> p i k", k=5),
                    axis=mybir.AxisListType.X, op=mybir.AluOpType.add)

                # pT = exp(lrelu(t))
                u = sb.tile([L, L], f32, tag="u")
                nc.scalar.activation(out=u[:], in_=t[:],
                                     func=mybir.ActivationFunctionType.Lrelu,
                                     alpha=NEG_SLOPE)
                pT = sb.tile([L, L], f32, tag="pT")
                nc.scalar.activation(out=pT[:], in_=u[:],
                                     func=mybir.ActivationFunctionType.Exp)

                # out rows + row-sums in one matmul (ones column)
                o_ps = ps_o.tile([L, D + 1], f32, tag="o_ps")
                nc.tensor.matmul(out=o_ps[:], lhsT=pT[:], rhs=h[:, 0:D + 1],
                                 start=True, stop=True)

                r = sb.tile([L, 1], f32, tag="r")
                nc.vector.reciprocal(r[:], o_ps[:, D:D + 1])
                o_sb = sb.tile([L, D], f32, tag="o_sb")
                nc.vector.tensor_scalar(out=o_sb[:], in0=o_ps[:, 0:D],
                                        scalar1=r[:, 0:1], scalar2=None,
                                        op0=mybir.AluOpType.mult)
                nc.sync.dma_start(out=out_d.ap()[n], in_=o_sb[:])

        if reps == 1:
            body()
        else:
            with tc.For_i(0, reps, 1) as iv:
                body(iv)

    nc.compile()
    return nc


_CACHED_NC = None


def _shard_inputs(inputs, adj, emb_table, a0, a1, a2, a3):
    inputs = np.asarray(inputs).astype(np.int32)
    adj = np.asarray(adj)
    emb_table = np.asarray(emb_table, dtype=np.float32)
    avecs = [np.asarray(a, dtype=np.float32) for a in (a0, a1, a2, a3)]

    emb_aug = np.concatenate(
        [emb_table, np.ones((V, 1), np.float32)], axis=1)   # [V, 129]
    a_pat = np.concatenate(
        [np.tile(a[:, None], (1, L)) for a in avecs], axis=1)  # [128, 400]

    in_maps = []
    for c in range(NCORES):
        sl = slice(c * BS, (c + 1) * BS)
        idx_c = np.ascontiguousarray(inputs[sl].T)                 # [L, BS]
        adj_c = np.ascontiguousarray(
            adj[sl].transpose(2, 0, 1)).astype(_BF16)              # [L,BS,L]
        in_maps.append(dict(emb=emb_aug, idx_t=idx_c, adj_t=adj_c,
                            a_pat=a_pat))
    return in_maps


def kernel(inputs, adj, mask_item, item, emb_table, a0, a1, a2, a3):
    """Full inputs in, full output out. mask_item/item are unused by the
    reference model's forward pass."""
    global _CACHED_NC
    if _CACHED_NC is None:
        _CACHED_NC = build_nc(reps=1)
    nc = _CACHED_NC

    in_maps = _shard_inputs(inputs, adj, emb_table, a0, a1, a2, a3)
    res = bass_utils.run_bass_kernel_spmd(nc, in_maps,
                                          core_ids=list(range(NCORES)))
    out = np.concatenate([np.asarray(res.results[c]["out"])
                          for c in range(NCORES)], axis=0)
    return out
